# revision 9
# baseline (speedup 1.0000x reference)
"""Trainium2 Bass kernel for link-prediction MLP (dense_mlp / ridge).

Model (per edge e with endpoints s, d):
    feat = emb[s] * emb[d]                  # [128]
    h1   = relu(feat @ W1 + b1)             # [32]
    h2   = relu(h1 @ W2 + b2)               # [32]
    pred = h2 @ W3 + b3                     # scalar

Distribution: 2-D shard over edges by (src-node-quarter, dst-node-quarter).
Nodes are split into 4 ranges of 25000; each edge falls in one of 16
(src_q, dst_q) buckets (~E/16 each for uniform ids). Each of the 8 cores
handles 2 buckets. Within a bucket, endpoint ids are local (< 25000), so the
Q7 `dma_gather` ucode (int16 indices, one instruction per few thousand rows)
can gather embedding rows at full DMA rate without per-row instruction
overhead.

Per core, per 4096-edge super-tile:
  - 2x dma_gather (src rows, dst rows) from the bucket's quarter tables,
  - DVE elementwise multiply -> fp16 features (edge-major),
  - PE transposes 128x128 chunks to feature-major (fp16, via identity),
  - PE matmuls for the 3 MLP layers (fp16 inputs, fp32 PSUM),
  - ACT relus with per-partition bias,
  - final layer uses lhsT=h2-chunk so predictions land 128-per-partition.

Host side only buckets/pads/permutes indices, slices the table into quarter
views, and casts the tiny MLP weights; all per-edge compute runs on device.
"""

import math

import numpy as np

import concourse.bass as bass
import concourse.mybir as mybir
import concourse.tile as tile
from concourse import bacc
from concourse.bass_utils import run_bass_kernel_spmd
from concourse.masks import make_identity

P = 128            # SBUF partitions
SUP = 4096         # edges per super-tile (one dma_gather pair)
CH = SUP // P      # 32 chunks of 128 edges
N_CORES = 8
EMB_DIM = 128
NUM_NODES = 100000
NQ = 4             # node quarters
QS = NUM_NODES // NQ  # 25000 rows per quarter (< 32768 so int16 indices work)
H = 32             # hidden width

F16 = mybir.dt.float16
F32 = mybir.dt.float32
I16 = mybir.dt.int16

# bucket order: core k handles buckets PAIRS[2k] and PAIRS[2k+1]
PAIRS = [(s, d) for s in range(NQ) for d in range(NQ)]


def build_program(nt: int):
    """SPMD program: two buckets, `nt` super-tiles each."""
    nc = bacc.Bacc(None, target_bir_lowering=False)

    qsrc = [
        nc.dram_tensor(f"qsrc{i}", [QS, EMB_DIM], F32, kind="ExternalInput")
        for i in range(2)
    ]
    qdst = [
        nc.dram_tensor(f"qdst{i}", [QS, EMB_DIM], F32, kind="ExternalInput")
        for i in range(2)
    ]
    isrc = nc.dram_tensor("isrc", [2 * nt, P, SUP // 16], I16, kind="ExternalInput")
    idst = nc.dram_tensor("idst", [2 * nt, P, SUP // 16], I16, kind="ExternalInput")
    w1 = nc.dram_tensor("w1", [EMB_DIM, H], F16, kind="ExternalInput")
    w2 = nc.dram_tensor("w2", [H, H], F16, kind="ExternalInput")
    w3 = nc.dram_tensor("w3", [H, 1], F16, kind="ExternalInput")
    b1 = nc.dram_tensor("b1", [H, 1], F32, kind="ExternalInput")
    b2 = nc.dram_tensor("b2", [H, 1], F32, kind="ExternalInput")
    b3 = nc.dram_tensor("b3", [P, 1], F32, kind="ExternalInput")
    out = nc.dram_tensor("out", [2 * nt, P, CH], F32, kind="ExternalOutput")

    relu = mybir.ActivationFunctionType.Relu

    with tile.TileContext(nc) as tc:
        with (
            tc.tile_pool(name="const", bufs=1) as cpool,
            tc.tile_pool(name="gpool", bufs=2) as gpool,
            tc.tile_pool(name="fpool", bufs=2) as fpool,
            tc.tile_pool(name="ffm", bufs=3) as ffm_pool,
            tc.tile_pool(name="hpool", bufs=3) as hpool,
            tc.tile_pool(name="ppool", bufs=2) as ppool,
            tc.tile_pool(name="idxp", bufs=3) as idxp,
            tc.tile_pool(name="psT", bufs=2, space="PSUM") as psT,
            tc.tile_pool(name="psH", bufs=2, space="PSUM") as psH,
            tc.tile_pool(name="psP", bufs=2, space="PSUM") as psP,
        ):
            ident = cpool.tile([P, P], F16)
            make_identity(nc, ident[:])
            w1s = cpool.tile([EMB_DIM, H], F16)
            nc.sync.dma_start(w1s[:], w1[:])
            w2s = cpool.tile([H, H], F16)
            nc.sync.dma_start(w2s[:], w2[:])
            w3s = cpool.tile([H, 1], F16)
            nc.sync.dma_start(w3s[:], w3[:])
            b1s = cpool.tile([H, 1], F32)
            nc.sync.dma_start(b1s[:], b1[:])
            b2s = cpool.tile([H, 1], F32)
            nc.sync.dma_start(b2s[:], b2[:])
            b3s = cpool.tile([P, 1], F32)
            nc.sync.dma_start(b3s[:], b3[:])

            for pair in range(2):
                qs_t, qd_t = qsrc[pair], qdst[pair]
                for t in range(nt):
                    tt = pair * nt + t
                    sidx = idxp.tile([P, SUP // 16], I16, tag="sidx")
                    nc.sync.dma_start(sidx[:], isrc[tt])
                    didx = idxp.tile([P, SUP // 16], I16, tag="didx")
                    nc.sync.dma_start(didx[:], idst[tt])

                    # gather slot i -> partition i%128, block i//128
                    gs = gpool.tile([P, CH * EMB_DIM], F32, tag="gs")
                    nc.gpsimd.dma_gather(
                        gs[:].rearrange("p (b f) -> p b f", b=CH),
                        qs_t[:],
                        sidx[:],
                        SUP,
                        SUP,
                        EMB_DIM,
                        single_packet=False,
                    )
                    gd = gpool.tile([P, CH * EMB_DIM], F32, tag="gd")
                    nc.gpsimd.dma_gather(
                        gd[:].rearrange("p (b f) -> p b f", b=CH),
                        qd_t[:],
                        didx[:],
                        SUP,
                        SUP,
                        EMB_DIM,
                        single_packet=False,
                    )

                    # feat[p, c*128+f] = src*dst of edge slot (p, c)
                    feat = fpool.tile([P, SUP], F16)
                    nc.vector.tensor_mul(feat[:], gs[:], gd[:])

                    predp = psP.tile([P, CH], F32)
                    for q in range(CH // 4):  # 512-edge chunks
                        ft = psT.tile([P, 512], F16, tag="ft")
                        for cc in range(4):
                            c = q * 4 + cc
                            nc.tensor.transpose(
                                ft[:, cc * P : (cc + 1) * P],
                                feat[:, c * P : (c + 1) * P],
                                ident[:],
                            )
                        ffm = ffm_pool.tile([P, 512], F16)
                        nc.vector.tensor_copy(ffm[:], ft[:])
                        h1p = psH.tile([H, 512], F32, tag="h1p")
                        nc.tensor.matmul(
                            h1p[:], lhsT=w1s[:], rhs=ffm[:], start=True, stop=True
                        )
                        h1s = hpool.tile([H, 512], F16, tag="h1s")
                        nc.scalar.activation(h1s[:], h1p[:], relu, bias=b1s[:])
                        h2p = psH.tile([H, 512], F32, tag="h2p")
                        nc.tensor.matmul(
                            h2p[:], lhsT=w2s[:], rhs=h1s[:], start=True, stop=True
                        )
                        h2s = hpool.tile([H, 512], F16, tag="h2s")
                        nc.scalar.activation(h2s[:], h2p[:], relu, bias=b2s[:])
                        for cc in range(4):
                            c = q * 4 + cc
                            nc.tensor.matmul(
                                predp[:, c : c + 1],
                                lhsT=h2s[:, cc * P : (cc + 1) * P],
                                rhs=w3s[:],
                                start=True,
                                stop=True,
                            )
                    preds = ppool.tile([P, CH], F32)
                    nc.vector.tensor_add(
                        preds[:], predp[:], b3s[:].to_broadcast([P, CH])
                    )
                    nc.sync.dma_start(out[tt], preds[:])

    nc.finalize()
    return nc


_PROGRAM_CACHE: dict[int, object] = {}


def _get_program(nt: int):
    if nt not in _PROGRAM_CACHE:
        _PROGRAM_CACHE[nt] = build_program(nt)
    return _PROGRAM_CACHE[nt]


def _wrap_idx(local_ids, nt):
    """[nt*SUP] int16 -> [nt, 128, SUP//16] wrapped (idx k -> [k%16, k//16]),
    replicated 8x across partitions."""
    w = local_ids.reshape(nt, SUP // 16, 16).transpose(0, 2, 1)  # [nt, 16, SUP//16]
    return np.ascontiguousarray(np.tile(w, (1, 8, 1)))


def prepare(node_id, edge_label_index, emb_table, W1, b1, W2, b2, W3, b3):
    """Host-side sharding: bucket edges, build per-core input maps."""
    node_id = np.asarray(node_id)
    edge_label_index = np.asarray(edge_label_index)
    emb_table = np.ascontiguousarray(np.asarray(emb_table, dtype=np.float32))

    E = edge_label_index.shape[1]
    src_all = np.asarray(node_id[edge_label_index[0]], dtype=np.int64)
    dst_all = np.asarray(node_id[edge_label_index[1]], dtype=np.int64)

    # bucket edges by (src quarter, dst quarter)
    bucket = (src_all // QS) * NQ + (dst_all // QS)
    order = np.argsort(bucket, kind="stable")
    counts = np.bincount(bucket, minlength=NQ * NQ)
    starts = np.zeros(NQ * NQ + 1, dtype=np.int64)
    np.cumsum(counts, out=starts[1:])

    nt = max(1, math.ceil(int(counts.max()) / SUP))
    cap = nt * SUP

    quarters = [emb_table[i * QS : (i + 1) * QS] for i in range(NQ)]
    w1h = np.ascontiguousarray(np.asarray(W1, dtype=np.float16))
    w2h = np.ascontiguousarray(np.asarray(W2, dtype=np.float16))
    w3h = np.ascontiguousarray(np.asarray(W3, dtype=np.float16))
    b1c = np.ascontiguousarray(np.asarray(b1, dtype=np.float32).reshape(H, 1))
    b2c = np.ascontiguousarray(np.asarray(b2, dtype=np.float32).reshape(H, 1))
    b3c = np.ascontiguousarray(
        np.broadcast_to(np.asarray(b3, dtype=np.float32).reshape(1, 1), (P, 1)).copy()
    )

    in_maps = []
    edge_pos = []  # per core: list of 2 arrays of original edge positions
    for k in range(N_CORES):
        m = {"w1": w1h, "w2": w2h, "w3": w3h, "b1": b1c, "b2": b2c, "b3": b3c}
        isrc = np.zeros((2, cap), dtype=np.int16)
        idst = np.zeros((2, cap), dtype=np.int16)
        pos_pair = []
        for j in range(2):
            bq = 2 * k + j
            s_q, d_q = PAIRS[bq]
            pos = order[starts[bq] : starts[bq + 1]]
            pos_pair.append(pos)
            isrc[j, : len(pos)] = (src_all[pos] - s_q * QS).astype(np.int16)
            idst[j, : len(pos)] = (dst_all[pos] - d_q * QS).astype(np.int16)
            m[f"qsrc{j}"] = quarters[s_q]
            m[f"qdst{j}"] = quarters[d_q]
        m["isrc"] = _wrap_idx(isrc.reshape(2 * nt, SUP), 2 * nt).reshape(
            2 * nt, P, SUP // 16
        )
        m["idst"] = _wrap_idx(idst.reshape(2 * nt, SUP), 2 * nt).reshape(
            2 * nt, P, SUP // 16
        )
        edge_pos.append(pos_pair)
        in_maps.append(m)

    return {"in_maps": in_maps, "edge_pos": edge_pos, "nt": nt, "E": E}


def unshard(prep, results):
    """results: list of per-core {"out": [2*nt, P, CH]} -> full [E] preds."""
    nt = prep["nt"]
    preds = np.empty(prep["E"], dtype=np.float32)
    for k in range(N_CORES):
        o = results[k]["out"]
        for j in range(2):
            pos = prep["edge_pos"][k][j]
            flat = (
                o[j * nt : (j + 1) * nt].transpose(0, 2, 1).reshape(-1)
            )  # edge order t*SUP + c*128 + p
            preds[pos] = flat[: len(pos)]
    return preds


def kernel(node_id, edge_label_index, emb_table, W1, b1, W2, b2, W3, b3):
    prep = prepare(node_id, edge_label_index, emb_table, W1, b1, W2, b2, W3, b3)
    nc = _get_program(prep["nt"])
    res = run_bass_kernel_spmd(
        nc, prep["in_maps"], core_ids=list(range(N_CORES)), trace=False
    )
    return unshard(prep, res.results)
